# revision 37
# baseline (speedup 1.0000x reference)
"""Trainium2 Bass kernel for nn_PairwisePredictionHead.

Math (reference):
  xd = x @ W_down.T + b_down             # [L, 128]
  q, k = xd[:, :64], xd[:, 64:]
  h[i,j,:] = W1p @ (q_j*k_i) + W1d @ (q_j - k_i) + b1    # [L, L, 128]
  g = gelu_exact(h)
  out = W2 @ LN(g) + b2                   # [L, L, 64]

Sharding: row-shard i across 8 cores (96 rows each). Each core gets the full
q-side (all 768 j) plus its own 96 k-rows; cores are independent (no
collectives), outputs concatenated on host.

Per-core algorithm (h on partitions, j on free; i processed in pairs, with
the stats/normalize tail batched over 8 pairs):
  - lhsT_i = [[W1p.T * k_i[:,None]] ; W1d.T]  (bf16; tops for a pair built in
    one GpSimd op into a [128,2,128] tile; W1d.T bottoms are static)
  - psum1[h, j] = lhsT_i.T @ [q.T; q.T]       (PE bf16, software-pipelined:
    MM1 of pair b+1 is issued before MM2 of pair b to keep the PE stream hot)
  - g = Gelu(psum1 + (b1 - W1d@k_i))          (ACT, bf16 out)
  - g2 = g*g                                  (DVE 2x bf16)
  - MM2 per 128-j chunk c (pair slot t): po[j, 512t+66c+0:65] accumulates
    g_c.T @ [W2z.T | 1/128]; col 512t+66c+65 = g2_c.T @ [1/128]
    W2z = (W2*ln_g) - rowmean: zero-mean rows absorb LN's mean subtraction.
    The 1/128 scaling folds the mean division into the weights, so
    mu = po[..,64], m2 = po[..,65] directly.
  - po psum -> SBUF bf16 copy (split ACT/DVE) frees PSUM after ~2 pairs and
    lets the whole tail run batched: r = rsqrt(m2 - mu^2 + eps) via clamp,
    DVE reciprocal, cubic seed in u = 1/v, one Newton step -- 11 ops on
    [128, 96] once per 8 pairs (no gpsimd pow, no ACT table switch).
  - out = r * po_main  (DVE/GpSimd split, bf16, DMA to HBM); the batched
    tail ops are interleaved into the next batch's per-pair stream so no
    engine sees a monolithic multi-microsecond block.
  - +b2 (= W2@ln_b + b2, handled generally for nonzero ln_b) added on host.
"""

import os
from contextlib import ExitStack

import numpy as np
import ml_dtypes

import concourse.bass as bass
import concourse.mybir as mybir
import concourse.tile as tile
from concourse import bacc
from concourse.bass_utils import run_bass_kernel_spmd

F32 = mybir.dt.float32
F32R = mybir.dt.float32r
BF16 = mybir.dt.bfloat16
ALU = mybir.AluOpType
AF = mybir.ActivationFunctionType

B, L, D = 1, 768, 1024
DP, H, NB = 128, 128, 64
NCORES = 8
ROWS = L // NCORES  # 96 pair-grid rows per core
P = 128
EPS = 1e-5

# rsqrt(v) approximation: clamp v to [VLO, VHI], u = 1/v, cubic seed in u,
# one Newton step.  Max rel err 1.6e-2 at range edges, ~3e-3 typical.
VLO, VHI = 0.13, 3.3
SC0, SC1, SC2, SC3 = 0.33759062, 0.78789633, -0.13786155, 0.01057634


def _build(nc):
    qqh = nc.dram_tensor("qqh", [P, L], BF16, kind="ExternalInput")
    kTh = nc.dram_tensor("kTh", [64, ROWS], BF16, kind="ExternalInput")
    b1ch = nc.dram_tensor("b1ch", [P, ROWS], F32, kind="ExternalInput")
    W1pT = nc.dram_tensor("W1pT", [64, P], BF16, kind="ExternalInput")
    W1dT = nc.dram_tensor("W1dT", [64, P], BF16, kind="ExternalInput")
    W2zTe = nc.dram_tensor("W2zTe", [P, 65], BF16, kind="ExternalInput")
    out = nc.dram_tensor("out", [ROWS, L, NB], BF16, kind="ExternalOutput")

    with tile.TileContext(nc) as tc, ExitStack() as ctx:
        const = ctx.enter_context(tc.tile_pool(name="const", bufs=1))
        work = ctx.enter_context(tc.tile_pool(name="work", bufs=3))
        outp = ctx.enter_context(tc.tile_pool(name="outp", bufs=3))
        statsp = ctx.enter_context(tc.tile_pool(name="statsp", bufs=3))
        pp1 = ctx.enter_context(tc.tile_pool(name="pp1", bufs=2, space="PSUM"))
        ppo = ctx.enter_context(tc.tile_pool(name="ppo", bufs=2, space="PSUM"))

        # ---- constants into SBUF (q/k/b1c precomputed on host) ----
        # dependency-ordered and spread across DMA queues: lhsT needs
        # kT/W1pT first, MM1 needs qq + lt2 bottoms, gelu needs b1c,
        # MM2 needs W2zTe last.
        kT_sb = const.tile([64, ROWS], BF16)
        nc.sync.dma_start(out=kT_sb, in_=kTh[:])
        W1pT_sb = const.tile([64, P], BF16)
        nc.scalar.dma_start(out=W1pT_sb, in_=W1pT[:])
        qq = const.tile([P, L], BF16)
        nc.scalar.dma_start(out=qq, in_=qqh[:])
        lt2_t = [const.tile([P, 2, P], BF16, tag=f"lt2{t}", name=f"lt2{t}")
                 for t in range(2)]
        for t in range(2):
            for s in range(2):
                eng = nc.sync if (t + s) % 2 == 0 else nc.scalar
                eng.dma_start(out=lt2_t[t][64:128, s, :], in_=W1dT[:])
        b1c = const.tile([P, ROWS], F32)
        nc.scalar.dma_start(out=b1c, in_=b1ch[:])
        W2zTe_sb = const.tile([P, 65], BF16)
        nc.sync.dma_start(out=W2zTe_sb, in_=W2zTe[:])

        # ---- software-pipelined main loop over pairs of i ----
        # Per pair: lhsT(b+1) -> MM1(b+1) -> gelu(b+1) -> MM2(b) -> g2(b+1)
        # -> DMA po2 PSUM->SBUF (frees PSUM fast).  The entire stats/rsqrt/
        # scale tail runs once per BATCH of 8 pairs on [128, 96]-shaped SBUF
        # data, amortizing the per-op overheads 8x and letting GpSimd help.
        NP = ROWS // 2       # 48 pairs
        BP = 16              # max pairs per tail batch
        TC = BP * 12         # 192 (t,c) blocks per full batch
        BATCH_END = {15: 16, 31: 16, 39: 8, 43: 4, 47: 4}

        def emit_lhs_mm1_gelu(b):
            ii0 = 2 * b
            lt2 = lt2_t[b % 2]
            nc.gpsimd.tensor_tensor(
                lt2[0:64, :, :],
                W1pT_sb[:, None, :].broadcast_to((64, 2, P)),
                kT_sb[:, ii0:ii0 + 2, None].broadcast_to((64, 2, P)),
                ALU.mult)
            gs = []
            for t in range(2):
                p1 = pp1.tile([P, 1024], F32, tag="p1", name="p1")
                nc.tensor.matmul(p1[:, 0:512], lt2[:, t, :], qq[:, 0:512],
                                 start=True, stop=True)
                nc.tensor.matmul(p1[:, 512:768], lt2[:, t, :], qq[:, 512:768],
                                 start=True, stop=True)
                g = work.tile([P, L], BF16, tag=f"g{t}", name="g")
                nc.scalar.activation(g, p1[:, 0:768], AF.Gelu,
                                     bias=b1c[:, ii0 + t:ii0 + t + 1])
                gs.append(g)
            return gs

        def emit_g2(gs):
            g2s = []
            for t in range(2):
                g2 = work.tile([P, L], BF16, tag=f"g2{t}", name="g2")
                nc.vector.tensor_tensor(g2, gs[t], gs[t], ALU.mult)
                g2s.append(g2)
            return g2s

        def emit_mm2_copy(b, gs, g2s, po_sb, boff):
            po2 = ppo.tile([P, 1024], F32, tag="po", name="po")
            for t in range(2):
                for c in range(6):
                    base = 512 * t + 66 * c
                    nc.tensor.matmul(po2[:, base:base + 65],
                                     gs[t][:, c * 128:(c + 1) * 128], W2zTe_sb,
                                     start=(c == 0), stop=False)
                    nc.tensor.matmul(po2[:, base + 65:base + 66],
                                     g2s[t][:, c * 128:(c + 1) * 128],
                                     W2zTe_sb[:, 64:65],
                                     start=False, stop=(c == 5))
            pov = po2[:].rearrange("p (t x) -> p t x", t=2)[:, :, 0:396]
            pov = pov.rearrange("p t (c w) -> p t c w", w=66)
            j = boff * 12
            nc.scalar.activation(po_sb[:, j:j + 6, :], pov[:, 0], AF.Identity)
            nc.vector.tensor_copy(po_sb[:, j + 6:j + 12, :], pov[:, 1])

        def tail_ops(bstart, bp, po_sb):
            """Closure list for batch B's stats/rsqrt/scale tail; the caller
            interleaves these into the next batch's per-pair stream so no
            engine sees a multi-microsecond monolithic block."""
            nblk = 12 * bp
            po_main = po_sb[:, 0:nblk, 0:64]
            muv = po_sb[:, 0:nblk, 64]
            m2v = po_sb[:, 0:nblk, 65]
            S = [P, nblk]
            st = {}

            def alloc(nm):
                t = statsp.tile(S, F32, tag=nm, name=nm)
                st[nm] = t
                return t

            def alloc_bf(nm):
                t = statsp.tile(S, BF16, tag=nm, name=nm)
                st[nm] = t
                return t

            ops = []
            ops.append(lambda: nc.vector.tensor_tensor(
                alloc("mu2"), muv, muv, ALU.mult))
            ops.append(lambda: nc.vector.scalar_tensor_tensor(
                alloc("veps"), m2v, EPS, st["mu2"][:], ALU.add, ALU.subtract))
            ops.append(lambda: nc.vector.tensor_scalar(
                alloc("vc"), st["veps"][:], VLO, VHI, ALU.max, ALU.min))
            ops.append(lambda: nc.vector.reciprocal(alloc("u"), st["vc"][:]))
            # cubic seed r0b = ((SC3*u + SC2)*u + SC1)*u + SC0
            ops.append(lambda: nc.vector.tensor_scalar(
                alloc("s1"), st["u"][:], SC3, SC2, ALU.mult, ALU.add))
            ops.append(lambda: nc.vector.tensor_tensor(
                alloc("s2"), st["s1"][:], st["u"][:], ALU.mult))
            ops.append(lambda: nc.vector.scalar_tensor_tensor(
                alloc("r0"), st["s2"][:], SC1, st["u"][:], ALU.add, ALU.mult))
            ops.append(lambda: nc.vector.tensor_scalar_add(
                alloc("r0b"), st["r0"][:], SC0))
            # Newton: r1 = r0b * (1.5 - 0.5 * vc * r0b^2)
            ops.append(lambda: nc.vector.tensor_tensor(
                alloc("t1"), st["r0b"][:], st["r0b"][:], ALU.mult))
            ops.append(lambda: nc.vector.scalar_tensor_tensor(
                alloc("w1"), st["t1"][:], -0.5, st["vc"][:],
                ALU.mult, ALU.mult))
            ops.append(lambda: nc.vector.scalar_tensor_tensor(
                alloc("r1"), st["w1"][:], 1.5, st["r0b"][:],
                ALU.add, ALU.mult))
            o2 = outp.tile([P, nblk, NB], BF16, tag="o2", name="o2")

            def o2_dve(g0, g1):
                rb = st["r1"][:, g0:g1, None].broadcast_to([P, g1 - g0, NB])
                nc.vector.tensor_tensor(o2[:, g0:g1, :], po_main[:, g0:g1, :],
                                        rb, ALU.mult)

            def o2_gps(g0, g1):
                rb = st["r1"][:, g0:g1, None].broadcast_to([P, g1 - g0, NB])
                nc.gpsimd.tensor_tensor(o2[:, g0:g1, :], po_main[:, g0:g1, :],
                                        rb, ALU.mult)

            r0lo = 2 * bstart
            nr = 2 * bp
            if bp >= 8:
                # dve 2/3 in bp/2 slices, gpsimd 1/3 in bp/4 slices
                dv = nblk * 2 // 3
                nd = bp // 2
                for g0 in range(0, dv, dv // nd):
                    ops.append(lambda g0=g0: o2_dve(g0, g0 + dv // nd))
                ng = bp // 4
                gstep = (nblk - dv) // ng
                for g0 in range(dv, nblk, gstep):
                    ops.append(lambda g0=g0: o2_gps(g0, g0 + gstep))
                ops.append(lambda: nc.sync.dma_start(
                    out=out[r0lo:r0lo + nr].rearrange(
                        "r (c p) n -> p (r c) n", p=P),
                    in_=o2))
            else:
                # short final batches: keep o2 off the slow Q7 path and
                # overlap the second half's scale with the first half's DMA
                h = nblk // 2
                ops.append(lambda: o2_dve(0, h))
                ops.append(lambda: nc.sync.dma_start(
                    out=out[r0lo:r0lo + bp].rearrange(
                        "r (c p) n -> p (r c) n", p=P),
                    in_=o2[:, 0:h, :]))
                ops.append(lambda: o2_dve(h, nblk))
                ops.append(lambda: nc.sync.dma_start(
                    out=out[r0lo + bp:r0lo + nr].rearrange(
                        "r (c p) n -> p (r c) n", p=P),
                    in_=o2[:, h:nblk, :]))
            return ops

        po_sbs = [const.tile([P, TC, 66], BF16, tag=f"posb{x}",
                             name=f"posb{x}") for x in range(2)]

        pending = []
        gs = emit_lhs_mm1_gelu(0)
        g2s = emit_g2(gs)
        bstart, bidx = 0, 0
        for b in range(NP):
            nxt_gs = emit_lhs_mm1_gelu(b + 1) if b + 1 < NP else None
            emit_mm2_copy(b, gs, g2s, po_sbs[bidx % 2], b - bstart)
            if nxt_gs is not None:
                nxt_g2s = emit_g2(nxt_gs)
            # interleave ~4 deferred tail ops of the previous batch per pair
            for _ in range(4):
                if pending:
                    pending.pop(0)()
            if b in BATCH_END:
                pending.extend(tail_ops(bstart, BATCH_END[b],
                                        po_sbs[bidx % 2]))
                bstart, bidx = b + 1, bidx + 1
            if nxt_gs is not None:
                gs, g2s = nxt_gs, nxt_g2s
        for op in pending:
            op()


def host_prep(x, W_down, b_down, W1, b1, ln_g, ln_b, W2, b2):
    f32 = np.float32
    bf16 = ml_dtypes.bfloat16
    xd = x[0].astype(f32) @ W_down.T.astype(f32) + b_down.astype(f32)
    q, k = xd[:, :64], xd[:, 64:]            # [L, 64] each
    common = {
        "qqh": np.ascontiguousarray(np.concatenate([q.T, q.T], 0).astype(bf16)),
        "W1pT": np.ascontiguousarray(W1[:, :64].T.astype(bf16)),
        "W1dT": np.ascontiguousarray(W1[:, 64:].T.astype(bf16)),
    }
    W2g = W2.astype(np.float64) * ln_g.astype(np.float64)[None, :]
    W2z = W2g - W2g.mean(axis=1, keepdims=True)
    W2zTe = np.concatenate([W2z.T, np.full((P, 1), 1.0 / 128.0)], axis=1)
    common["W2zTe"] = np.ascontiguousarray(W2zTe.astype(bf16))
    cvec = W2.astype(np.float64) @ ln_b.astype(np.float64) + b2.astype(np.float64)
    W1d = W1[:, 64:].astype(f32)
    b1cfull = b1.astype(f32)[:, None] - W1d @ k.T        # [128, L]
    kbf = k.T.astype(bf16)                               # [64, L]
    return common, kbf, b1cfull, cvec.astype(f32)


def kernel(x, W_down, b_down, W1, b1, ln_g, ln_b, W2, b2):
    x = np.asarray(x)
    common, kbf, b1cfull, cvec = host_prep(
        x, np.asarray(W_down), np.asarray(b_down), np.asarray(W1),
        np.asarray(b1), np.asarray(ln_g), np.asarray(ln_b), np.asarray(W2),
        np.asarray(b2))

    nc = bacc.Bacc("TRN2")
    _build(nc)
    nc.finalize()

    in_maps = []
    for core in range(NCORES):
        m = dict(common)
        i0 = core * ROWS
        m["kTh"] = np.ascontiguousarray(kbf[:, i0:i0 + ROWS])
        m["b1ch"] = np.ascontiguousarray(b1cfull[:, i0:i0 + ROWS])
        in_maps.append(m)

    trace = os.environ.get("KERNEL_TRACE", "0") == "1"
    tmpdir = os.environ.get("KERNEL_TMPDIR") or None
    res = run_bass_kernel_spmd(nc, in_maps, core_ids=list(range(NCORES)),
                               trace=trace, tmpdir=tmpdir)
    if trace and res.exec_time_ns is not None:
        print(f"HW exec time: {res.exec_time_ns} ns")
    outs = [res.results[c]["out"] for c in range(NCORES)]
    full = np.concatenate(outs, axis=0)  # [768, 768, 64] bf16
    full = full.astype(np.float32) + cvec[None, None, :]
    return full[None]


# revision 38
# speedup vs baseline: 1.0576x; 1.0576x over previous
"""Trainium2 Bass kernel for nn_PairwisePredictionHead.

Math (reference):
  xd = x @ W_down.T + b_down             # [L, 128]
  q, k = xd[:, :64], xd[:, 64:]
  h[i,j,:] = W1p @ (q_j*k_i) + W1d @ (q_j - k_i) + b1    # [L, L, 128]
  g = gelu_exact(h)
  out = W2 @ LN(g) + b2                   # [L, L, 64]

Sharding: row-shard i across 8 cores (96 rows each). Each core gets the full
q-side (all 768 j) plus its own 96 k-rows; cores are independent (no
collectives), outputs concatenated on host.

Per-core algorithm (h on partitions, j on free; i processed in pairs, with
the stats/normalize tail batched over 8 pairs):
  - lhsT_i = [[W1p.T * k_i[:,None]] ; W1d.T]  (bf16; tops for a pair built in
    one GpSimd op into a [128,2,128] tile; W1d.T bottoms are static)
  - psum1[h, j] = lhsT_i.T @ [q.T; q.T]       (PE bf16, software-pipelined:
    MM1 of pair b+1 is issued before MM2 of pair b to keep the PE stream hot)
  - g = Gelu(psum1 + (b1 - W1d@k_i))          (ACT, bf16 out)
  - g2 = g*g                                  (DVE 2x bf16)
  - MM2 per 128-j chunk c (pair slot t): po[j, 512t+66c+0:65] accumulates
    g_c.T @ [W2z.T | 1/128]; col 512t+66c+65 = g2_c.T @ [1/128]
    W2z = (W2*ln_g) - rowmean: zero-mean rows absorb LN's mean subtraction.
    The 1/128 scaling folds the mean division into the weights, so
    mu = po[..,64], m2 = po[..,65] directly.
  - po psum -> SBUF bf16 copy (split ACT/DVE) frees PSUM after ~2 pairs and
    lets the whole tail run batched: r = rsqrt(m2 - mu^2 + eps) via clamp,
    DVE reciprocal, cubic seed in u = 1/v, one Newton step -- 11 ops on
    [128, 96] once per 8 pairs (no gpsimd pow, no ACT table switch).
  - out = r * po_main  (DVE/GpSimd split, bf16, DMA to HBM); the batched
    tail ops are interleaved into the next batch's per-pair stream so no
    engine sees a monolithic multi-microsecond block.
  - +b2 (= W2@ln_b + b2, handled generally for nonzero ln_b) added on host.
"""

import os
from contextlib import ExitStack

import numpy as np
import ml_dtypes

import concourse.bass as bass
import concourse.mybir as mybir
import concourse.tile as tile
from concourse import bacc
from concourse.bass_utils import run_bass_kernel_spmd

F32 = mybir.dt.float32
F32R = mybir.dt.float32r
BF16 = mybir.dt.bfloat16
ALU = mybir.AluOpType
AF = mybir.ActivationFunctionType

B, L, D = 1, 768, 1024
DP, H, NB = 128, 128, 64
NCORES = 8
ROWS = L // NCORES  # 96 pair-grid rows per core
P = 128
EPS = 1e-5

# rsqrt(v) approximation: clamp v to [VLO, VHI], u = 1/v, cubic seed in u,
# one Newton step.  Max rel err 1.6e-2 at range edges, ~3e-3 typical.
VLO, VHI = 0.13, 3.3
SC0, SC1, SC2, SC3 = 0.33759062, 0.78789633, -0.13786155, 0.01057634


def _build(nc):
    qqh = nc.dram_tensor("qqh", [P, L], BF16, kind="ExternalInput")
    kTh = nc.dram_tensor("kTh", [64, ROWS], BF16, kind="ExternalInput")
    b1ch = nc.dram_tensor("b1ch", [P, ROWS], F32, kind="ExternalInput")
    W1pT = nc.dram_tensor("W1pT", [64, P], BF16, kind="ExternalInput")
    W1dT = nc.dram_tensor("W1dT", [64, P], BF16, kind="ExternalInput")
    W2zTe = nc.dram_tensor("W2zTe", [P, 65], BF16, kind="ExternalInput")
    out = nc.dram_tensor("out", [ROWS, L, NB], BF16, kind="ExternalOutput")

    with tile.TileContext(nc) as tc, ExitStack() as ctx:
        const = ctx.enter_context(tc.tile_pool(name="const", bufs=1))
        work = ctx.enter_context(tc.tile_pool(name="work", bufs=3))
        outp = ctx.enter_context(tc.tile_pool(name="outp", bufs=3))
        statsp = ctx.enter_context(tc.tile_pool(name="statsp", bufs=3))
        pp1 = ctx.enter_context(tc.tile_pool(name="pp1", bufs=2, space="PSUM"))
        ppo = ctx.enter_context(tc.tile_pool(name="ppo", bufs=2, space="PSUM"))

        # ---- constants into SBUF (q/k/b1c precomputed on host) ----
        # dependency-ordered and spread across DMA queues: lhsT needs
        # kT/W1pT first, MM1 needs qq + lt2 bottoms, gelu needs b1c,
        # MM2 needs W2zTe last.
        kT_sb = const.tile([64, ROWS], BF16)
        nc.sync.dma_start(out=kT_sb, in_=kTh[:])
        W1pT_sb = const.tile([64, P], BF16)
        nc.scalar.dma_start(out=W1pT_sb, in_=W1pT[:])
        qq = const.tile([P, L], BF16)
        nc.scalar.dma_start(out=qq, in_=qqh[:])
        lt2_t = [const.tile([P, 2, P], BF16, tag=f"lt2{t}", name=f"lt2{t}")
                 for t in range(2)]
        for t in range(2):
            for s in range(2):
                eng = nc.sync if (t + s) % 2 == 0 else nc.scalar
                eng.dma_start(out=lt2_t[t][64:128, s, :], in_=W1dT[:])
        b1c = const.tile([P, ROWS], F32)
        nc.scalar.dma_start(out=b1c, in_=b1ch[:])
        W2zTe_sb = const.tile([P, 65], BF16)
        nc.sync.dma_start(out=W2zTe_sb, in_=W2zTe[:])

        # ---- software-pipelined main loop over pairs of i ----
        # Per pair: lhsT(b+1) -> MM1(b+1) -> gelu(b+1) -> MM2(b) -> g2(b+1)
        # -> DMA po2 PSUM->SBUF (frees PSUM fast).  The entire stats/rsqrt/
        # scale tail runs once per BATCH of 8 pairs on [128, 96]-shaped SBUF
        # data, amortizing the per-op overheads 8x and letting GpSimd help.
        NP = ROWS // 2       # 48 pairs
        BP = 8               # max pairs per tail batch
        TC = BP * 12         # 96 (t,c) blocks per full batch
        BATCH_END = {7: 8, 15: 8, 23: 8, 31: 8, 39: 8, 43: 4, 47: 4}

        def emit_lhs_mm1_gelu(b):
            ii0 = 2 * b
            lt2 = lt2_t[b % 2]
            nc.gpsimd.tensor_tensor(
                lt2[0:64, :, :],
                W1pT_sb[:, None, :].broadcast_to((64, 2, P)),
                kT_sb[:, ii0:ii0 + 2, None].broadcast_to((64, 2, P)),
                ALU.mult)
            gs = []
            for t in range(2):
                p1 = pp1.tile([P, 1024], F32, tag="p1", name="p1")
                nc.tensor.matmul(p1[:, 0:512], lt2[:, t, :], qq[:, 0:512],
                                 start=True, stop=True)
                nc.tensor.matmul(p1[:, 512:768], lt2[:, t, :], qq[:, 512:768],
                                 start=True, stop=True)
                g = work.tile([P, L], BF16, tag=f"g{t}", name="g")
                nc.scalar.activation(g, p1[:, 0:768], AF.Gelu,
                                     bias=b1c[:, ii0 + t:ii0 + t + 1])
                gs.append(g)
            return gs

        def emit_g2(gs):
            g2s = []
            for t in range(2):
                g2 = work.tile([P, L], BF16, tag=f"g2{t}", name="g2")
                nc.vector.tensor_tensor(g2, gs[t], gs[t], ALU.mult)
                g2s.append(g2)
            return g2s

        def emit_mm2_copy(b, gs, g2s, po_sb, boff):
            po2 = ppo.tile([P, 1024], F32, tag="po", name="po")
            for t in range(2):
                for c in range(6):
                    base = 512 * t + 66 * c
                    nc.tensor.matmul(po2[:, base:base + 65],
                                     gs[t][:, c * 128:(c + 1) * 128], W2zTe_sb,
                                     start=(c == 0), stop=False)
                    nc.tensor.matmul(po2[:, base + 65:base + 66],
                                     g2s[t][:, c * 128:(c + 1) * 128],
                                     W2zTe_sb[:, 64:65],
                                     start=False, stop=(c == 5))
            pov = po2[:].rearrange("p (t x) -> p t x", t=2)[:, :, 0:396]
            pov = pov.rearrange("p t (c w) -> p t c w", w=66)
            j = boff * 12
            nc.scalar.activation(po_sb[:, j:j + 6, :], pov[:, 0], AF.Identity)
            nc.vector.tensor_copy(po_sb[:, j + 6:j + 12, :], pov[:, 1])

        def tail_ops(bstart, bp, po_sb):
            """Closure list for batch B's stats/rsqrt/scale tail; the caller
            interleaves these into the next batch's per-pair stream so no
            engine sees a multi-microsecond monolithic block."""
            nblk = 12 * bp
            po_main = po_sb[:, 0:nblk, 0:64]
            muv = po_sb[:, 0:nblk, 64]
            m2v = po_sb[:, 0:nblk, 65]
            S = [P, nblk]
            st = {}

            def alloc(nm):
                t = statsp.tile(S, F32, tag=nm, name=nm)
                st[nm] = t
                return t

            def alloc_bf(nm):
                t = statsp.tile(S, BF16, tag=nm, name=nm)
                st[nm] = t
                return t

            ops = []
            ops.append(lambda: nc.vector.tensor_tensor(
                alloc("mu2"), muv, muv, ALU.mult))
            ops.append(lambda: nc.vector.scalar_tensor_tensor(
                alloc("veps"), m2v, EPS, st["mu2"][:], ALU.add, ALU.subtract))
            ops.append(lambda: nc.vector.tensor_scalar(
                alloc("vc"), st["veps"][:], VLO, VHI, ALU.max, ALU.min))
            ops.append(lambda: nc.vector.reciprocal(alloc("u"), st["vc"][:]))
            # cubic seed r0b = ((SC3*u + SC2)*u + SC1)*u + SC0
            ops.append(lambda: nc.vector.tensor_scalar(
                alloc("s1"), st["u"][:], SC3, SC2, ALU.mult, ALU.add))
            ops.append(lambda: nc.vector.tensor_tensor(
                alloc("s2"), st["s1"][:], st["u"][:], ALU.mult))
            ops.append(lambda: nc.vector.scalar_tensor_tensor(
                alloc("r0"), st["s2"][:], SC1, st["u"][:], ALU.add, ALU.mult))
            ops.append(lambda: nc.vector.tensor_scalar_add(
                alloc("r0b"), st["r0"][:], SC0))
            # Newton: r1 = r0b * (1.5 - 0.5 * vc * r0b^2)
            ops.append(lambda: nc.vector.tensor_tensor(
                alloc("t1"), st["r0b"][:], st["r0b"][:], ALU.mult))
            ops.append(lambda: nc.vector.scalar_tensor_tensor(
                alloc("w1"), st["t1"][:], -0.5, st["vc"][:],
                ALU.mult, ALU.mult))
            ops.append(lambda: nc.vector.scalar_tensor_tensor(
                alloc("r1"), st["w1"][:], 1.5, st["r0b"][:],
                ALU.add, ALU.mult))
            o2 = outp.tile([P, nblk, NB], BF16, tag="o2", name="o2")

            def o2_dve(g0, g1):
                rb = st["r1"][:, g0:g1, None].broadcast_to([P, g1 - g0, NB])
                nc.vector.tensor_tensor(o2[:, g0:g1, :], po_main[:, g0:g1, :],
                                        rb, ALU.mult)

            def o2_gps(g0, g1):
                rb = st["r1"][:, g0:g1, None].broadcast_to([P, g1 - g0, NB])
                nc.gpsimd.tensor_tensor(o2[:, g0:g1, :], po_main[:, g0:g1, :],
                                        rb, ALU.mult)

            r0lo = 2 * bstart
            nr = 2 * bp
            if bp == BP:
                dv = nblk * 2 // 3
                for g0 in range(0, dv, dv // 4):
                    ops.append(lambda g0=g0: o2_dve(g0, g0 + dv // 4))
                gstep = (nblk - dv) // 2
                ops.append(lambda: o2_gps(dv, dv + gstep))
                ops.append(lambda: o2_gps(dv + gstep, nblk))
                ops.append(lambda: nc.sync.dma_start(
                    out=out[r0lo:r0lo + nr].rearrange(
                        "r (c p) n -> p (r c) n", p=P),
                    in_=o2))
            else:
                # short final batches: keep o2 off the slow Q7 path and
                # overlap the second half's scale with the first half's DMA
                h = nblk // 2
                ops.append(lambda: o2_dve(0, h))
                ops.append(lambda: nc.sync.dma_start(
                    out=out[r0lo:r0lo + bp].rearrange(
                        "r (c p) n -> p (r c) n", p=P),
                    in_=o2[:, 0:h, :]))
                ops.append(lambda: o2_dve(h, nblk))
                ops.append(lambda: nc.sync.dma_start(
                    out=out[r0lo + bp:r0lo + nr].rearrange(
                        "r (c p) n -> p (r c) n", p=P),
                    in_=o2[:, h:nblk, :]))
            return ops

        po_sbs = [const.tile([P, TC, 66], BF16, tag=f"posb{x}",
                             name=f"posb{x}") for x in range(2)]

        pending = []
        gs = emit_lhs_mm1_gelu(0)
        g2s = emit_g2(gs)
        bstart, bidx = 0, 0
        for b in range(NP):
            nxt_gs = emit_lhs_mm1_gelu(b + 1) if b + 1 < NP else None
            emit_mm2_copy(b, gs, g2s, po_sbs[bidx % 2], b - bstart)
            if nxt_gs is not None:
                nxt_g2s = emit_g2(nxt_gs)
            # interleave ~4 deferred tail ops of the previous batch per pair
            for _ in range(4):
                if pending:
                    pending.pop(0)()
            if b in BATCH_END:
                pending.extend(tail_ops(bstart, BATCH_END[b],
                                        po_sbs[bidx % 2]))
                bstart, bidx = b + 1, bidx + 1
            if nxt_gs is not None:
                gs, g2s = nxt_gs, nxt_g2s
        for op in pending:
            op()


def host_prep(x, W_down, b_down, W1, b1, ln_g, ln_b, W2, b2):
    f32 = np.float32
    bf16 = ml_dtypes.bfloat16
    xd = x[0].astype(f32) @ W_down.T.astype(f32) + b_down.astype(f32)
    q, k = xd[:, :64], xd[:, 64:]            # [L, 64] each
    common = {
        "qqh": np.ascontiguousarray(np.concatenate([q.T, q.T], 0).astype(bf16)),
        "W1pT": np.ascontiguousarray(W1[:, :64].T.astype(bf16)),
        "W1dT": np.ascontiguousarray(W1[:, 64:].T.astype(bf16)),
    }
    W2g = W2.astype(np.float64) * ln_g.astype(np.float64)[None, :]
    W2z = W2g - W2g.mean(axis=1, keepdims=True)
    W2zTe = np.concatenate([W2z.T, np.full((P, 1), 1.0 / 128.0)], axis=1)
    common["W2zTe"] = np.ascontiguousarray(W2zTe.astype(bf16))
    cvec = W2.astype(np.float64) @ ln_b.astype(np.float64) + b2.astype(np.float64)
    W1d = W1[:, 64:].astype(f32)
    b1cfull = b1.astype(f32)[:, None] - W1d @ k.T        # [128, L]
    kbf = k.T.astype(bf16)                               # [64, L]
    return common, kbf, b1cfull, cvec.astype(f32)


def kernel(x, W_down, b_down, W1, b1, ln_g, ln_b, W2, b2):
    x = np.asarray(x)
    common, kbf, b1cfull, cvec = host_prep(
        x, np.asarray(W_down), np.asarray(b_down), np.asarray(W1),
        np.asarray(b1), np.asarray(ln_g), np.asarray(ln_b), np.asarray(W2),
        np.asarray(b2))

    nc = bacc.Bacc("TRN2")
    _build(nc)
    nc.finalize()

    in_maps = []
    for core in range(NCORES):
        m = dict(common)
        i0 = core * ROWS
        m["kTh"] = np.ascontiguousarray(kbf[:, i0:i0 + ROWS])
        m["b1ch"] = np.ascontiguousarray(b1cfull[:, i0:i0 + ROWS])
        in_maps.append(m)

    trace = os.environ.get("KERNEL_TRACE", "0") == "1"
    tmpdir = os.environ.get("KERNEL_TMPDIR") or None
    res = run_bass_kernel_spmd(nc, in_maps, core_ids=list(range(NCORES)),
                               trace=trace, tmpdir=tmpdir)
    if trace and res.exec_time_ns is not None:
        print(f"HW exec time: {res.exec_time_ns} ns")
    outs = [res.results[c]["out"] for c in range(NCORES)]
    full = np.concatenate(outs, axis=0)  # [768, 768, 64] bf16
    full = full.astype(np.float32) + cvec[None, None, :]
    return full[None]
